# revision 13
# baseline (speedup 1.0000x reference)
"""Trainium2 Bass kernel: LayerNorm + bottleneck MLP (768 -> 64 -> 768, both ReLU).

Strategy
--------
Data-parallel over 8 NeuronCores: 8192 tokens per core, weights replicated.

The TensorEngine contracts along the partition axis, so the kernel works in a
feature-major ("transposed") layout: the host pre-transposes each token shard
to xT [768, 8192] (cast to bf16, halving HBM traffic) so features land on SBUF
partitions with plain contiguous DMAs, and un-transposes the returned
outT [768, 8192].  LayerNorm is folded into the first matmul:

  pre_h[t,m] = r_t*xw[t,m] - (sum_t/768)*a[m]*r_t... algebraically:
  h[t,m]   = relu( r_t * (xw[t,m] + nega[m]*sum_t) + b[m] )
  xw       = x @ (w1*gamma).T   (6 K-chunk bf16 matmuls on raw x, fp32 PSUM;
                                 stationary col 64 = ones -> PSUM row 64 = sum)
  sumsq    = ones-stationary matmul group on x^2 (separate PSUM row)
  nega[m]  = -sum_c gamma_c*w1[m,c]/768,  b[m] = sum_c beta_c*w1[m,c]  (host)
  r_t      = rsqrt(var+eps) via Ln/Exp on ScalarE (Rsqrt activation is banned)
  sum_t and r_t are broadcast across partitions with a single K=2 selector
  matmul (ones outer product), then applied with fused DVE ops.

Matmuls run in bf16 (1 cycle/row; fp32 is 4x slower and fp32r trips the BIR
verifier's rounding rule), accumulating in fp32 PSUM.  I/O is bf16; the host
upcasts the result.  Error vs the f32 reference is ~1e-3, well under the 2e-2
gate.
"""

import math
import os
import sys

import numpy as np

os.environ.setdefault("MYCRO_LOCAL_CACHE", "1")

if not any("trn_rl_repo" in p for p in sys.path):
    for _p in ("/opt/trn_rl_repo", "/root/.axon_site/_ro/trn_rl_repo"):
        if os.path.isdir(_p):
            sys.path.insert(0, _p)
            break

N_CORES = 8
N_TOKENS = 65536
C_IN = 768
C_MID = 64
KCH = C_IN // 128  # 6 contraction chunks
EPS = 1e-5
TOK_PER_CORE = N_TOKENS // N_CORES  # 8192
TILE_T = 1024  # tokens per SBUF tile (one DMA in / one DMA out)
HALF_T = 512  # tokens per PSUM pass (one fp32 PSUM bank)

LAST_RESULTS = None
_NC_CACHE = {}


def build_nc(tok_per_core=TOK_PER_CORE, tile_t=TILE_T):
    import concourse.tile as tile
    from concourse import bacc, mybir
    from contextlib import ExitStack

    f32 = mybir.dt.float32
    bf16 = mybir.dt.bfloat16
    AF = mybir.ActivationFunctionType
    OP = mybir.AluOpType

    T = tok_per_core
    n_tiles = T // tile_t
    n_half = tile_t // HALF_T
    assert T % tile_t == 0 and tile_t % HALF_T == 0

    nc = bacc.Bacc()
    x_ext = nc.declare_dram_parameter("xT", [C_IN, T], bf16, isOutput=False)
    w1e_ext = nc.declare_dram_parameter("w1e", [C_IN, C_MID + 1], bf16, isOutput=False)
    w2t_ext = nc.declare_dram_parameter("w2t", [C_MID, C_IN], bf16, isOutput=False)
    na_ext = nc.declare_dram_parameter("nega", [C_MID, 1], f32, isOutput=False)
    b_ext = nc.declare_dram_parameter("bvec", [C_MID, 1], f32, isOutput=False)
    o_ext = nc.declare_dram_parameter("out", [C_IN, T], bf16, isOutput=True)

    # feature row c = 128*k + p  ->  partition p, chunk k
    x_v = x_ext[:].rearrange("(k p) t -> p k t", p=128)
    o_v = o_ext[:].rearrange("(k p) t -> p k t", p=128)

    with tile.TileContext(nc) as tc, ExitStack() as ctx:
        singles = ctx.enter_context(tc.tile_pool(name="singles", bufs=1))
        xpool = ctx.enter_context(tc.tile_pool(name="xp", bufs=2))
        opool = ctx.enter_context(tc.tile_pool(name="op", bufs=2))
        sqpool = ctx.enter_context(tc.tile_pool(name="sqp", bufs=3))
        hpool = ctx.enter_context(tc.tile_pool(name="hp", bufs=2))
        prepool = ctx.enter_context(tc.tile_pool(name="prep", bufs=2))
        bcpool = ctx.enter_context(tc.tile_pool(name="bcp", bufs=2))
        srpool = ctx.enter_context(tc.tile_pool(name="srp", bufs=2))
        stpool = ctx.enter_context(tc.tile_pool(name="stp", bufs=6))
        php = ctx.enter_context(tc.tile_pool(name="php", bufs=2, space="PSUM"))
        pstp = ctx.enter_context(tc.tile_pool(name="pstp", bufs=1, space="PSUM"))
        pbcp = ctx.enter_context(tc.tile_pool(name="pbcp", bufs=1, space="PSUM"))
        pop = ctx.enter_context(tc.tile_pool(name="pop", bufs=3, space="PSUM"))

        # ---- constants (loaded once) ----
        w1e_sb = singles.tile([128, KCH, C_MID + 1], bf16)
        nc.sync.dma_start(
            out=w1e_sb[:], in_=w1e_ext[:].rearrange("(k p) m -> p k m", p=128)
        )
        w2t_sb = singles.tile([C_MID, C_IN], bf16)
        nc.sync.dma_start(out=w2t_sb[:], in_=w2t_ext[:])
        nega_sb = singles.tile([C_MID, 1], f32)
        nc.sync.dma_start(out=nega_sb[:], in_=na_ext[:])
        bcol_sb = singles.tile([C_MID, 1], f32)
        nc.sync.dma_start(out=bcol_sb[:], in_=b_ext[:])
        ones1 = singles.tile([128, 1], bf16)
        nc.vector.memset(ones1[:], 1.0)
        onesrow = singles.tile([1, C_MID], bf16)
        nc.vector.memset(onesrow[:], 1.0)
        eps_t = singles.tile([1, 1], f32)
        nc.vector.memset(eps_t[:], EPS)

        for it in range(n_tiles):
            x_sb = xpool.tile([128, KCH, tile_t], bf16)
            nc.sync.dma_start(out=x_sb[:], in_=x_v[:, :, it * tile_t:(it + 1) * tile_t])
            o_sb = opool.tile([128, KCH, tile_t], bf16)
            for ih in range(n_half):
                t0 = ih * HALF_T
                # sumsq group -> row 0 of its own PSUM tile
                pst = pstp.tile([1, HALF_T], f32)
                sq_all = sqpool.tile([128, KCH, HALF_T], bf16)
                xs_all = x_sb[:, :, t0:t0 + HALF_T]
                nc.vector.tensor_mul(sq_all[:], xs_all, xs_all)
                for k in range(KCH):
                    nc.tensor.matmul(
                        pst[0:1, :],
                        lhsT=ones1[:],
                        rhs=sq_all[:, k, :],
                        start=(k == 0),
                        stop=(k == KCH - 1),
                    )
                # xw group -> rows 0:64 = x @ w1g.T, row 64 = sum_c x
                ph = php.tile([C_MID + 1, HALF_T], f32)
                for k in range(KCH):
                    nc.tensor.matmul(
                        ph[:, :],
                        lhsT=w1e_sb[:, k, :],
                        rhs=x_sb[:, k, t0:t0 + HALF_T],
                        start=(k == 0),
                        stop=(k == KCH - 1),
                    )

                # ---- LayerNorm statistics ----
                sum_sb = srpool.tile([1, HALF_T], bf16)
                nc.vector.tensor_copy(out=sum_sb[:], in_=ph[C_MID:C_MID + 1, :])
                u_sb = stpool.tile([1, HALF_T], f32)
                nc.scalar.activation(
                    out=u_sb[:], in_=ph[C_MID:C_MID + 1, :], func=AF.Square,
                    scale=1.0 / math.sqrt(C_IN),
                )  # u = sum^2/768
                q_sb = stpool.tile([1, HALF_T], f32)
                nc.vector.tensor_tensor(
                    out=q_sb[:], in0=pst[0:1, :], in1=u_sb[:], op=OP.subtract
                )  # q = sumsq - sum^2/768 = 768*var
                l_sb = stpool.tile([1, HALF_T], f32)
                nc.scalar.activation(
                    out=l_sb[:], in_=q_sb[:], func=AF.Ln,
                    bias=eps_t[:], scale=1.0 / C_IN,
                )  # l = ln(var + eps)
                r_sb = srpool.tile([1, HALF_T], bf16)
                nc.scalar.activation(
                    out=r_sb[:], in_=l_sb[:], func=AF.Exp, scale=-0.5
                )  # r = rsqrt(var + eps)

                # broadcast both stats across the 64 h-partitions, side by side
                # in the free dim: pbc[:, 0, :] = sum, pbc[:, 1, :] = r
                pbc = pbcp.tile([C_MID, 2, HALF_T], f32)
                nc.tensor.matmul(
                    pbc[:, 0, :], lhsT=onesrow[:], rhs=sum_sb[:],
                    start=True, stop=True,
                )
                nc.tensor.matmul(
                    pbc[:, 1, :], lhsT=onesrow[:], rhs=r_sb[:],
                    start=True, stop=True,
                )
                bc_sb = bcpool.tile([C_MID, 2, HALF_T], f32)
                nc.vector.tensor_copy(out=bc_sb[:], in_=pbc[:])

                # pre1 = nega*sum_b + xw ; pre2 = pre1 * r_b ; h = relu(pre2+b)
                pre_sb = prepool.tile([C_MID, HALF_T], f32)
                nc.vector.scalar_tensor_tensor(
                    out=pre_sb[:], in0=bc_sb[:, 0, :], scalar=nega_sb[:],
                    in1=ph[0:C_MID, :], op0=OP.mult, op1=OP.add,
                )
                pre2_sb = prepool.tile([C_MID, HALF_T], f32)
                nc.vector.tensor_tensor(
                    out=pre2_sb[:], in0=pre_sb[:], in1=bc_sb[:, 1, :],
                    op=OP.mult,
                )
                h_sb = hpool.tile([C_MID, HALF_T], bf16)
                nc.vector.tensor_scalar(
                    out=h_sb[:], in0=pre2_sb[:],
                    scalar1=bcol_sb[:], scalar2=0.0,
                    op0=OP.add, op1=OP.max,
                )  # h = relu(pre2 + b)

                # ---- second matmul + fused ReLU copy-out ----
                for k in range(KCH):
                    po = pop.tile([128, HALF_T], f32)
                    nc.tensor.matmul(
                        po[:],
                        lhsT=w2t_sb[:, k * 128:(k + 1) * 128],
                        rhs=h_sb[:],
                        start=True,
                        stop=True,
                    )
                    # all copy-outs on ScalarE: the out-DMA is issued from
                    # the ScalarE queue, so same-engine program order replaces
                    # cross-engine semaphore waits (walrus limits wait slots)
                    nc.scalar.activation(
                        out=o_sb[:, k, t0:t0 + HALF_T], in_=po[:], func=AF.Relu
                    )
            nc.scalar.dma_start(
                out=o_v[:, :, it * tile_t:(it + 1) * tile_t], in_=o_sb[:]
            )
    nc.finalize()  # run the Bacc pipeline (wait splitting, reg alloc, ...)
    return nc


def _get_nc(tok_per_core, tile_t=TILE_T):
    key = (tok_per_core, tile_t)
    if key not in _NC_CACHE:
        _NC_CACHE[key] = build_nc(tok_per_core, tile_t)
    return _NC_CACHE[key]


def prep_weights(w1, w2, gamma, beta):
    import ml_dtypes

    w1 = np.asarray(w1, dtype=np.float32)  # [64, 768] (out, in)
    w2 = np.asarray(w2, dtype=np.float32)  # [768, 64] (out, in)
    gamma = np.asarray(gamma, dtype=np.float32)
    beta = np.asarray(beta, dtype=np.float32)

    w1g = w1 * gamma[None, :]  # [64, 768]
    w1e = np.empty((C_IN, C_MID + 1), np.float32)
    w1e[:, :C_MID] = w1g.T
    w1e[:, C_MID] = 1.0
    w1e = w1e.astype(ml_dtypes.bfloat16)
    w2t = np.ascontiguousarray(w2.T).astype(ml_dtypes.bfloat16)  # [64, 768]
    a = w1g.sum(axis=1)  # a[m] = sum_c gamma_c*w1[m,c]
    nega = np.ascontiguousarray((-a / C_IN)[:, None])  # [64, 1] f32
    bvec = np.ascontiguousarray((w1 @ beta)[:, None])  # [64, 1] f32
    return w1e, w2t, nega, bvec


def kernel(x, w1, w2, gamma, beta):
    global LAST_RESULTS
    import ml_dtypes
    from concourse.bass_utils import run_bass_kernel_spmd

    x = np.asarray(x, dtype=np.float32)
    assert x.shape == (N_TOKENS, C_IN), x.shape
    w1e, w2t, nega, bvec = prep_weights(w1, w2, gamma, beta)

    tok = TOK_PER_CORE
    in_maps = []
    for s in range(N_CORES):
        xs = np.ascontiguousarray(x[s * tok:(s + 1) * tok].T).astype(
            ml_dtypes.bfloat16
        )  # [768, 8192] bf16
        in_maps.append(
            {"xT": xs, "w1e": w1e, "w2t": w2t, "nega": nega, "bvec": bvec}
        )

    nc = _get_nc(tok)
    br = run_bass_kernel_spmd(nc, in_maps, core_ids=list(range(N_CORES)))
    LAST_RESULTS = br

    out = np.empty((N_TOKENS, C_IN), np.float32)
    for s in range(N_CORES):
        out[s * tok:(s + 1) * tok] = br.results[s]["out"].astype(np.float32).T
    return out
